# revision 11
# baseline (speedup 1.0000x reference)
"""Trainium2 Bass kernel for nn_Attention_49907519979595 (Bahdanau-style attention).

Math (per batch b):
    q      = query @ Wq.T + bq                      [H]
    r_s    = Wr @ ref_s + br                        [S, H]
    logit  = V . tanh(q + r_s)                      [S]
    w      = softmax(logit)                         [S]
    expected = sum_s w_s r_s = Wr @ (sum_s w_s ref_s) + br     (softmax weights sum to 1)
    result = concat(output, expected) @ Wo.T + bo   [H]

Key implementation ideas:
  - Data-parallel over batch: 8 cores x 8 batches each. Each core streams its
    32 MiB slice of `ref` exactly once (memory-bound regime).
  - No-max softmax: |logit| <= ||V||_1 <= 16, so exp() is computed directly
    in f32 without the max subtraction; no online rescaling needed.
  - The weighted sum runs over *raw* ref (linearity), so the kernel keeps ref
    in SBUF in two layouts: natural [s, h] (bf16, for the weighted-sum matmul)
    and transposed [h, s] (bf16, for the r matmul) produced by SB->SB DMA
    transpose.
  - tanh( r + (q+br) ) runs on ScalarE with the per-partition bias port.
  - logits come out with s on partitions (lhsT = tanh tile), so exp() and the
    weighted accumulation are fully partition-parallel.
  - Final projection folds Wr into Wo: result = output @ WoA.T
    + (acc/Z) @ (WoB@Wr).T + (WoB@br + bo); all computed on-device in f32.
"""

import os
import sys

import numpy as np

sys.path.insert(0, "/opt/trn_rl_repo")

H = 256
B = 64
S = 4096
N_CORES = 8
B_CORE = B // N_CORES  # 8
S_TILE = 512
N_STILES = S // S_TILE  # 8
N_J = S_TILE // 128  # 4 s-chunks of 128 per tile

_nc_cache = {}


def build_nc():
    import concourse.bass as bass
    import concourse.bacc as bacc
    import concourse.tile as tile
    from concourse import masks, mybir

    f32 = mybir.dt.float32
    bf16 = mybir.dt.bfloat16
    AF = mybir.ActivationFunctionType

    nc = bacc.Bacc("TRN2", debug=False)

    ref = nc.dram_tensor("ref", [B_CORE, S, H], f32, kind="ExternalInput").ap()
    query = nc.dram_tensor("query", [B_CORE, H], f32, kind="ExternalInput").ap()
    out_prev = nc.dram_tensor("out_prev", [B_CORE, H], f32, kind="ExternalInput").ap()
    Wq = nc.dram_tensor("Wq", [H, H], f32, kind="ExternalInput").ap()
    bq = nc.dram_tensor("bq", [H], f32, kind="ExternalInput").ap()
    Wr = nc.dram_tensor("Wr", [H, H], f32, kind="ExternalInput").ap()
    br = nc.dram_tensor("br", [H], f32, kind="ExternalInput").ap()
    Wo = nc.dram_tensor("Wo", [H, 2 * H], f32, kind="ExternalInput").ap()
    bo = nc.dram_tensor("bo", [H], f32, kind="ExternalInput").ap()
    V = nc.dram_tensor("V", [H], f32, kind="ExternalInput").ap()
    result = nc.dram_tensor("result", [B_CORE, H], f32, kind="ExternalOutput").ap()
    DBG = bool(int(os.environ.get("KERNEL_DEBUG", "0")))
    if DBG:
        dbg_accs = nc.dram_tensor("dbg_accs", [B_CORE, H + 1], f32, kind="ExternalOutput").ap()
        dbg_bias = nc.dram_tensor("dbg_bias", [128, 2, B_CORE], f32, kind="ExternalOutput").ap()
        dbg_nat = nc.dram_tensor("dbg_nat", [128, N_J, H], bf16, kind="ExternalOutput").ap()
        dbg_refT = nc.dram_tensor("dbg_refT", [128, 2, S_TILE], bf16, kind="ExternalOutput").ap()
        dbg_tanh = nc.dram_tensor("dbg_tanh", [128, 2, S_TILE], bf16, kind="ExternalOutput").ap()
        dbg_e = nc.dram_tensor("dbg_e", [128, N_J], bf16, kind="ExternalOutput").ap()

    with tile.TileContext(nc) as tc:
        with (
            tc.tile_pool(name="const", bufs=1) as const,
            tc.tile_pool(name="work", bufs=3) as work,
            tc.tile_pool(name="reftp", bufs=3) as reftp,
            tc.tile_pool(name="tanhp", bufs=3) as tanhp,
            tc.tile_pool(name="small", bufs=2) as small,
            tc.tile_pool(name="psum_r", bufs=4, space="PSUM") as psum_r,
            tc.tile_pool(name="psum_s", bufs=2, space="PSUM") as psum_s,
            tc.tile_pool(name="psum_acc", bufs=2, space="PSUM") as psum_acc,
        ):
            # ---------------- prologue: weights & biases ----------------
            ident = const.tile([128, 128], f32, name="ident")
            masks.make_identity(nc, ident[:])

            ones_col = const.tile([128, 1], bf16, name="ones_col")
            nc.vector.memset(ones_col[:], 1.0)

            # Column-layout vectors [128, 2]: element (c*128+p) -> [p, c]
            def load_col(vec_ap, name):
                t = const.tile([128, 2], f32, name=name)
                nc.sync.dma_start(t[:], vec_ap.rearrange("(c p) -> p c", p=128))
                return t

            bq_col = load_col(bq, "bq_col")
            br_col = load_col(br, "br_col")
            bo_col = load_col(bo, "bo_col")
            V_f32 = load_col(V, "V_f32")
            V_col = const.tile([128, 2], bf16, name="V_col")
            nc.vector.tensor_copy(V_col[:], V_f32[:])



            # Natural-layout weight rows in SBUF: X_nat[c] = X[c*128:(c+1)*128, :]
            def load_rows(mat_ap, ncols, name):
                t = const.tile([128, 2, ncols], f32, name=name)
                nc.sync.dma_start(
                    t[:], mat_ap.rearrange("(c p) n -> p c n", p=128)
                )
                return t

            Wq_nat = load_rows(Wq, H, "Wq_nat")
            Wr_nat = load_rows(Wr, H, "Wr_nat")
            Wo_nat = load_rows(Wo, 2 * H, "Wo_nat")

            # Transpose helper: returns SBUF tile [128, 2, 256] holding M.T
            # (rows of M.T chunked: out[c][p, q_chunk*128+q] = M[q_chunk*128+q, c*128+p])
            def transpose_256(nat, out_dtype, name, col0=0):
                t = const.tile([128, 2, H], out_dtype, name=name)
                for c in range(2):  # chunk of M.T rows (= M cols)
                    for g in range(2):  # chunk of M rows
                        tp = psum_s.tile([128, 256], f32, name=f"{name}_tp", tag="ps")[:, :128]
                        nc.tensor.transpose(
                            tp[:], nat[:, g, col0 + c * 128 : col0 + (c + 1) * 128], ident[:]
                        )
                        nc.scalar.copy(t[:, c, g * 128 : (g + 1) * 128], tp[:])
                return t

            WqT = transpose_256(Wq_nat, f32, "WqT")
            WrT = transpose_256(Wr_nat, bf16, "WrT")
            WoAT = transpose_256(Wo_nat, f32, "WoAT", col0=0)
            WoBT = transpose_256(Wo_nat, f32, "WoBT", col0=H)

            # MT[cm] = (WoB @ Wr).T rows: MT[cm][p, :] = sum_h Wr[h, cm*128+p] * WoB[:, h]
            MT = const.tile([128, 2, H], f32, name="MT")
            for cm in range(2):
                mt_ps = psum_s.tile([128, H], f32, name="mt_ps", tag="ps")
                for ck in range(2):
                    nc.tensor.matmul(
                        mt_ps[:],
                        Wr_nat[:, ck, cm * 128 : (cm + 1) * 128],
                        WoBT[:, ck, :],
                        start=(ck == 0),
                        stop=(ck == 1),
                    )
                nc.scalar.copy(MT[:, cm, :], mt_ps[:])

            # c_col[co] = WoB @ br + bo  (column layout [128, 2])
            c_col = const.tile([128, 2], f32, name="c_col")
            for co in range(2):
                c_ps = psum_s.tile([128, 1], f32, name="c_ps", tag="ps")
                for ck in range(2):
                    nc.tensor.matmul(
                        c_ps[:],
                        WoBT[:, ck, co * 128 : (co + 1) * 128],
                        br_col[:, ck : ck + 1],
                        start=(ck == 0),
                        stop=(ck == 1),
                    )
                nc.scalar.activation(
                    c_col[:, co : co + 1], c_ps[:], AF.Identity, bias=bo_col[:, co : co + 1]
                )

            # qT + (bq+br): bias_sb[c][p, b] = q[b][c*128+p] + bq[...] + br[...]
            query_sb = const.tile([B_CORE, H], f32, name="query_sb")
            nc.sync.dma_start(query_sb[:], query)
            queryT = const.tile([128, 2, B_CORE], f32, name="queryT")
            for c in range(2):
                qt_ps = psum_s.tile([128, B_CORE], f32, name="qt_ps", tag="ps")
                nc.tensor.transpose(
                    qt_ps[:], query_sb[:, c * 128 : (c + 1) * 128], ident[:B_CORE, :B_CORE]
                )
                nc.scalar.copy(queryT[:, c, :], qt_ps[:])
            bias_sb = const.tile([128, 2, B_CORE], f32, name="bias_sb")
            for c in range(2):
                q_ps = psum_s.tile([128, B_CORE], f32, name="q_ps", tag="ps")
                for ck in range(2):
                    nc.tensor.matmul(
                        q_ps[:],
                        WqT[:, ck, c * 128 : (c + 1) * 128],
                        queryT[:, ck, :],
                        start=(ck == 0),
                        stop=(ck == 1),
                    )
                nc.scalar.activation(
                    bias_sb[:, c, :], q_ps[:], AF.Identity, bias=bq_col[:, c : c + 1]
                )
                nc.scalar.activation(
                    bias_sb[:, c, :], bias_sb[:, c, :], AF.Identity,
                    bias=br_col[:, c : c + 1]
                )

            # ---------------- main loop ----------------
            accs_list = [
                const.tile([1, H + 1], f32, name=f"accs{b}", tag=f"accs{b}")
                for b in range(B_CORE)
            ]

            for b in range(B_CORE):
                for t in range(N_STILES):
                    s0 = t * S_TILE
                    # natural layout (cast to bf16): nat[p, j, h] = ref[b, s0 + j*128 + p, h]
                    nat = work.tile([128, N_J, H], bf16, name="nat")
                    nc.gpsimd.dma_start(
                        nat[:],
                        ref[b, s0 : s0 + S_TILE, :].rearrange("(j p) h -> p j h", p=128),
                    )
                    # transposed layout: refT[hh][p, j*128+q] = nat[q, j, hh*128+p]
                    refT = reftp.tile([128, 2, S_TILE], bf16, name="refT")
                    for hh in range(2):
                        for j in range(N_J):
                            nc.sync.dma_start(
                                refT[:, hh, j * 128 : (j + 1) * 128],
                                nat[:, j, hh * 128 : (hh + 1) * 128],
                                transpose=True,
                            )
                    # r.T in PSUM: r_ps[hh][p, s] = sum_hin Wr[hh*128+p, hin] refT[hin, s]
                    tanh_sb = tanhp.tile([128, 2, S_TILE], bf16, name="tanh_sb")
                    for hh in range(2):
                        r_ps = psum_r.tile([128, S_TILE], f32, name="r_ps")
                        for ck in range(2):
                            nc.tensor.matmul(
                                r_ps[:],
                                WrT[:, ck, hh * 128 : (hh + 1) * 128],
                                refT[:, ck, :],
                                start=(ck == 0),
                                stop=(ck == 1),
                            )
                        nc.scalar.activation(
                            tanh_sb[:, hh, :],
                            r_ps[:],
                            AF.Tanh,
                            bias=bias_sb[:, hh, b : b + 1],
                        )
                    # logits: [128, j] with s = j*128 + p on partitions
                    lg_ps = psum_s.tile([128, N_J], f32, name="lg_ps", tag="ps")
                    for j in range(N_J):
                        for hh in range(2):
                            nc.tensor.matmul(
                                lg_ps[:, j : j + 1],
                                tanh_sb[:, hh, j * 128 : (j + 1) * 128],
                                V_col[:, hh : hh + 1],
                                start=(j == 0 and hh == 0),
                                stop=(j == N_J - 1 and hh == 1),
                                skip_group_check=True,
                            )
                    e_sb = small.tile([128, N_J], bf16, name="e_sb")
                    nc.scalar.activation(e_sb[:], lg_ps[:], AF.Exp)
                    if DBG and b == int(os.environ.get('KERNEL_DBG_B','3')) and t == int(os.environ.get('KERNEL_DBG_T','0')):
                        nc.gpsimd.dma_start(dbg_nat, nat[:])
                        nc.gpsimd.dma_start(dbg_refT, refT[:])
                        nc.gpsimd.dma_start(dbg_tanh, tanh_sb[:])
                        nc.gpsimd.dma_start(dbg_e, e_sb[:])
                    # weighted accumulation over raw ref + Z (per-tile PSUM group)
                    acc_ps = psum_acc.tile([1, H + 1], f32, name="acc_ps")
                    for j in range(N_J):
                        nc.tensor.matmul(
                            acc_ps[:, 0:H],
                            e_sb[:, j : j + 1],
                            nat[:, j, :],
                            start=(j == 0),
                            stop=False,
                            skip_group_check=True,
                        )
                        nc.tensor.matmul(
                            acc_ps[:, H : H + 1],
                            e_sb[:, j : j + 1],
                            ones_col[:],
                            start=False,
                            stop=(j == N_J - 1),
                            skip_group_check=True,
                        )
                    if t == 0:
                        nc.vector.tensor_copy(accs_list[b][:], acc_ps[:])
                    else:
                        nc.vector.tensor_add(accs_list[b][:], accs_list[b][:], acc_ps[:])

            # ---------------- epilogue ----------------
            if DBG:
                for b in range(B_CORE):
                    nc.gpsimd.dma_start(dbg_accs[b : b + 1, :], accs_list[b][:])
                nc.gpsimd.dma_start(dbg_bias, bias_sb[:])
            acc8 = small.tile([B_CORE, H + 1], f32, name="acc8")
            for b in range(B_CORE):
                nc.sync.dma_start(acc8[b : b + 1, :], accs_list[b][:])
            rz = small.tile([B_CORE, 1], f32, name="rz")
            nc.vector.reciprocal(rz[:], acc8[:, H : H + 1])
            u_sb = small.tile([B_CORE, H], f32, name="u_sb")
            nc.vector.tensor_scalar_mul(u_sb[:], acc8[:, 0:H], rz[:])

            outp_sb = small.tile([B_CORE, H], f32, name="outp_sb")
            nc.sync.dma_start(outp_sb[:], out_prev)

            uT = small.tile([128, 2, B_CORE], f32, name="uT")
            outT = small.tile([128, 2, B_CORE], f32, name="outT")
            for c in range(2):
                ut_ps = psum_s.tile([128, B_CORE], f32, name="ut_ps", tag="ps")
                nc.tensor.transpose(
                    ut_ps[:], u_sb[:, c * 128 : (c + 1) * 128], ident[:B_CORE, :B_CORE]
                )
                nc.vector.tensor_copy(uT[:, c, :], ut_ps[:])
                ot_ps = psum_s.tile([128, B_CORE], f32, name="ot_ps", tag="ps")
                nc.tensor.transpose(
                    ot_ps[:], outp_sb[:, c * 128 : (c + 1) * 128], ident[:B_CORE, :B_CORE]
                )
                nc.vector.tensor_copy(outT[:, c, :], ot_ps[:])

            res_sb = small.tile([B_CORE, H], f32, name="res_sb")
            for co in range(2):
                r2_ps = psum_s.tile([128, B_CORE], f32, name="r2_ps", tag="ps")
                for ck in range(2):
                    nc.tensor.matmul(
                        r2_ps[:],
                        WoAT[:, ck, co * 128 : (co + 1) * 128],
                        outT[:, ck, :],
                        start=(ck == 0),
                        stop=False,
                        skip_group_check=True,
                    )
                for ck in range(2):
                    nc.tensor.matmul(
                        r2_ps[:],
                        MT[:, ck, co * 128 : (co + 1) * 128],
                        uT[:, ck, :],
                        start=False,
                        stop=(ck == 1),
                        skip_group_check=True,
                    )
                resT_sb = small.tile([128, B_CORE], f32, name="resT_sb")
                nc.scalar.activation(
                    resT_sb[:], r2_ps[:], AF.Identity, bias=c_col[:, co : co + 1]
                )
                rb_ps = psum_s.tile([B_CORE, 128], f32, name="rb_ps", tag="ps")
                nc.tensor.transpose(rb_ps[:], resT_sb[:], ident[:])
                nc.vector.tensor_copy(res_sb[:, co * 128 : (co + 1) * 128], rb_ps[:])

            nc.sync.dma_start(result, res_sb[:])

    nc.compile()
    return nc


def _get_nc():
    if "nc" not in _nc_cache:
        _nc_cache["nc"] = build_nc()
    return _nc_cache["nc"]


def kernel(output, query, ref, Wq, bq, Wr, br, Wo, bo, V):
    from concourse.bass_utils import run_bass_kernel_spmd

    output = np.ascontiguousarray(np.asarray(output, dtype=np.float32))
    query = np.ascontiguousarray(np.asarray(query, dtype=np.float32))
    ref = np.ascontiguousarray(np.asarray(ref, dtype=np.float32))
    shared = {
        "Wq": np.ascontiguousarray(np.asarray(Wq, np.float32)),
        "bq": np.ascontiguousarray(np.asarray(bq, np.float32)),
        "Wr": np.ascontiguousarray(np.asarray(Wr, np.float32)),
        "br": np.ascontiguousarray(np.asarray(br, np.float32)),
        "Wo": np.ascontiguousarray(np.asarray(Wo, np.float32)),
        "bo": np.ascontiguousarray(np.asarray(bo, np.float32)),
        "V": np.ascontiguousarray(np.asarray(V, np.float32)),
    }

    nc = _get_nc()
    in_maps = []
    for c in range(N_CORES):
        sl = slice(c * B_CORE, (c + 1) * B_CORE)
        in_maps.append(
            {
                "ref": ref[sl],
                "query": query[sl],
                "out_prev": output[sl],
                **shared,
            }
        )

    trace = bool(int(os.environ.get("KERNEL_TRACE", "0")))
    res = run_bass_kernel_spmd(nc, in_maps, list(range(N_CORES)), trace=trace)
    if trace:
        kernel.last_exec_time_ns = res.exec_time_ns
        kernel.last_profile = res
    out = np.concatenate([res.results[c]["result"] for c in range(N_CORES)], axis=0)
    return out.reshape(B, 1, H)
